# revision 4
# baseline (speedup 1.0000x reference)
"""Trainium2 Bass kernel: 3D 'same' convolution (implicit GEMM, bf16).

Problem: x (4, 64, 24, 24, 24) f32, weight (1, 128, 1728) f32
         -> out (4, 128, 24, 24, 24) f32  (SAME conv3d, k=3)

Sharding (8 cores): batch (4) x z-halves (2). Each core computes
out[b, :, z0:z0+12] for its (b, zh) shard; no inter-core communication.

Per-core algorithm: tap-PAIR-packed implicit GEMM in bf16. The 27 taps
are packed two-per-matmul along the 128-partition contraction dim:
partitions 0-63 hold the zero-padded input window, partitions 64-127
hold the same window pre-shifted by a fixed tap offset. Three such
buffer sets (shifts (0,0,1), (0,1,0), (1,0,0)) cover 13 pairs; the
27th tap rides as a 14th matmul with zeroed upper weights. This halves
the serial matmul-column count vs. the 27x 64-deep formulation
(the PE array streams 1 column/cycle regardless of contraction depth).

Each output tile accumulates its 14 matmuls into a single PSUM bank
(no cross-bank add needed), evacuated with one ACT copy and DMA'd out.
Buffer sets are staged in two overlapping 8-plane z-chunks so the
second half's DMA hides under the first half's matmuls.
"""

import sys

if "/opt/trn_rl_repo" not in sys.path:
    sys.path.insert(0, "/opt/trn_rl_repo")

import numpy as np

CIN, COUT, K = 64, 128, 3
DHW = 24  # cubic spatial extent
ZS = 12  # z-planes per shard
NP = 14  # padded z-planes per shard window (ZS + 2 halo)
PW = 26  # padded y/x extent
N_CORES = 8

# tap-pair table: (set_idx, (dz, dy, dx) AP offset, lo tap, hi tap or None)
# set 0: upper half shifted (0,0,1); set 1: (0,1,0); set 2: (1,0,0)
PAIRS = (
    [(0, (dz, dy, 0), (dz, dy, 0), (dz, dy, 1)) for dz in range(3) for dy in range(3)]
    + [(0, (2, 2, 2), (2, 2, 2), None)]
    + [(1, (dz, 0, 2), (dz, 0, 2), (dz, 1, 2)) for dz in range(3)]
    + [(2, (0, 2, 2), (0, 2, 2), (1, 2, 2))]
)
N_SLOT = len(PAIRS)  # 14


def _build_program(loop_n=None):
    """Build the SPMD Bass program (one NeuronCore's view).

    loop_n: if set, wrap the whole body in a hardware For_i loop with
    that many iterations (used by test.py for wall-clock timing).
    """
    import concourse.tile as tile
    from concourse import bacc, mybir

    F32 = mybir.dt.float32
    BF16 = mybir.dt.bfloat16

    nc = bacc.Bacc("TRN2")
    s_in = [
        nc.declare_dram_parameter(f"s{k}", [128, NP, PW, PW], BF16, isOutput=False)
        for k in range(3)
    ]
    wk_in = nc.declare_dram_parameter("wk", [128, N_SLOT, 128], BF16, isOutput=False)
    y_out = nc.declare_dram_parameter("y", [128, ZS, DHW, DHW], F32, isOutput=True)

    with tile.TileContext(nc) as tc:
        with (
            tc.tile_pool(name="xw", bufs=1) as xw_pool,
            tc.tile_pool(name="ps", bufs=4, space="PSUM") as ps_pool,
            tc.tile_pool(name="ob", bufs=3) as ob_pool,
        ):

            def body(_iv=None):
                W = xw_pool.tile([128, N_SLOT, 128], BF16, name="W")
                nc.sync.dma_start(out=W[:], in_=wk_in[:])
                # per set: two overlapping 8-plane chunks
                SA = [xw_pool.tile([128, 8, PW, PW], BF16, name=f"SA{k}") for k in range(3)]
                SB = [xw_pool.tile([128, 8, PW, PW], BF16, name=f"SB{k}") for k in range(3)]
                for k in range(3):
                    nc.sync.dma_start(out=SA[k][:], in_=s_in[k][:, 0:8])
                for k in range(3):
                    nc.sync.dma_start(out=SB[k][:], in_=s_in[k][:, 6:14])

                # output tiles: ("plane", chunk, zoff, z) N=504 (21x24, 2D AP)
                #           or ("rem", chunk, zoff, None) N=432 (6x3x24, 3D)
                tiles = (
                    [("plane", SA, 0, z) for z in range(6)]
                    + [("rem", SA, 0, None)]
                    + [("plane", SB, 6, z) for z in range(6, 12)]
                    + [("rem", SB, 6, None)]
                )

                def rhs_ap(X, zoff, kind, z, dz, dy, dx):
                    if kind == "plane":
                        return X[:, z - zoff + dz, dy : dy + 21, dx : dx + 24]
                    return X[:, dz : dz + 6, 21 + dy : 24 + dy, dx : dx + 24]

                for kind, S, zoff, z in tiles:
                    n = 504 if kind == "plane" else 432
                    ps = ps_pool.tile([128, 512], F32, name="ps", tag="ps")
                    for s, (si, (dz, dy, dx), _lo, _hi) in enumerate(PAIRS):
                        nc.tensor.matmul(
                            ps[:, :n],
                            lhsT=W[:, s, :],
                            rhs=rhs_ap(S[si], zoff, kind, z, dz, dy, dx),
                            start=(s == 0),
                            stop=(s == N_SLOT - 1),
                        )
                    ob = ob_pool.tile([128, 512], F32, name="ob", tag="ob")
                    nc.scalar.copy(ob[:, :n], ps[:, :n])
                    if kind == "plane":
                        nc.sync.dma_start(out=y_out[:, z, 0:21, :], in_=ob[:, :n])
                    else:
                        # one DMA per z-plane: keeps each transfer one
                        # contiguous run per partition (descriptor-lean)
                        for j in range(6):
                            nc.sync.dma_start(
                                out=y_out[:, zoff + j, 21:24, :],
                                in_=ob[:, j * 72 : (j + 1) * 72],
                            )

            if loop_n is not None:
                with tc.For_i(0, loop_n, 1) as _i:
                    body(_i)
            else:
                body()

    nc.finalize()
    return nc


def _make_in_maps(x, weight):
    import ml_dtypes

    BF16 = ml_dtypes.bfloat16
    w = np.asarray(weight, np.float32).reshape(COUT, CIN, K, K, K)
    wk = np.zeros((128, N_SLOT, 128), BF16)
    for s, (_si, _off, lo, hi) in enumerate(PAIRS):
        wk[0:64, s, :] = w[:, :, lo[0], lo[1], lo[2]].T.astype(BF16)
        if hi is not None:
            wk[64:128, s, :] = w[:, :, hi[0], hi[1], hi[2]].T.astype(BF16)

    # upper-half shifts per buffer set
    SHIFTS = [(0, 0, 1), (0, 1, 0), (1, 0, 0)]

    in_maps = []
    for c in range(N_CORES):
        b, zh = divmod(c, 2)
        z0 = zh * ZS
        # 27^3 pad so +1 shifts stay in range (extra zero plane at 26)
        xpad = np.zeros((CIN, 27, 27, 27), BF16)
        xpad[:, 1:25, 1:25, 1:25] = x[b].astype(BF16)
        lo_win = xpad[:, z0 : z0 + NP, 0:PW, 0:PW]  # (64, 14, 26, 26)
        im = {"wk": wk}
        for k, (sz, sy, sx) in enumerate(SHIFTS):
            S = np.empty((128, NP, PW, PW), BF16)
            S[0:64] = lo_win
            S[64:128] = xpad[
                :, z0 + sz : z0 + sz + NP, sy : sy + PW, sx : sx + PW
            ]
            im[f"s{k}"] = S
        in_maps.append(im)
    return in_maps


def _gather(results):
    out = np.empty((4, COUT, DHW, DHW, DHW), np.float32)
    for c in range(N_CORES):
        b, zh = divmod(c, 2)
        out[b, :, zh * ZS : (zh + 1) * ZS] = results[c]["y"]
    return out


def kernel(x, weight):
    from concourse.bass_utils import run_bass_kernel_spmd

    x = np.asarray(x, np.float32)
    in_maps = _make_in_maps(x, weight)
    nc = _build_program()
    res = run_bass_kernel_spmd(nc, in_maps, list(range(N_CORES)))
    return _gather(res.results)


# revision 5
# speedup vs baseline: 1.1559x; 1.1559x over previous
"""Trainium2 Bass kernel: 3D 'same' convolution (implicit GEMM, bf16).

Problem: x (4, 64, 24, 24, 24) f32, weight (1, 128, 1728) f32
         -> out (4, 128, 24, 24, 24) f32  (SAME conv3d, k=3)

Sharding (8 cores): batch (4) x z-halves (2). Each core computes
out[b, :, z0:z0+12] for its (b, zh) shard; no inter-core communication.

Per-core algorithm: tap-PAIR-packed implicit GEMM in bf16. The 27 taps
are packed two-per-matmul along the 128-partition contraction dim:
partitions 0-63 hold the zero-padded input window, partitions 64-127
hold the same window pre-shifted by a fixed tap offset. Three such
buffer sets (shifts (0,0,1), (0,1,0), (1,0,0)) cover 13 pairs; the
27th tap rides as a 14th matmul with zeroed upper weights. This halves
the serial matmul-column count vs. the 27x 64-deep formulation
(the PE array streams 1 column/cycle regardless of contraction depth).

Each output tile accumulates its 14 matmuls into a single PSUM bank
(no cross-bank add needed), evacuated with one ACT copy and DMA'd out.
Buffer sets are staged in two overlapping 8-plane z-chunks so the
second half's DMA hides under the first half's matmuls.
"""

import sys

if "/opt/trn_rl_repo" not in sys.path:
    sys.path.insert(0, "/opt/trn_rl_repo")

import numpy as np

CIN, COUT, K = 64, 128, 3
DHW = 24  # cubic spatial extent
ZS = 12  # z-planes per shard
NP = 14  # padded z-planes per shard window (ZS + 2 halo)
PW = 26  # padded y/x extent
N_CORES = 8

# tap-pair table: (set_idx, (dz, dy, dx) AP offset, lo tap, hi tap or None)
# set 0: upper half shifted (0,0,1); set 1: (0,1,0); set 2: (1,0,0)
PAIRS = (
    [(0, (dz, dy, 0), (dz, dy, 0), (dz, dy, 1)) for dz in range(3) for dy in range(3)]
    + [(0, (2, 2, 2), (2, 2, 2), None)]
    + [(1, (dz, 0, 2), (dz, 0, 2), (dz, 1, 2)) for dz in range(3)]
    + [(2, (0, 2, 2), (0, 2, 2), (1, 2, 2))]
)
N_SLOT = len(PAIRS)  # 14


def _build_program(loop_n=None):
    """Build the SPMD Bass program (one NeuronCore's view).

    loop_n: if set, wrap the whole body in a hardware For_i loop with
    that many iterations (used by test.py for wall-clock timing).
    """
    import concourse.tile as tile
    from concourse import bacc, mybir

    F32 = mybir.dt.float32
    BF16 = mybir.dt.bfloat16

    nc = bacc.Bacc("TRN2")
    s_in = [
        nc.declare_dram_parameter(f"s{k}", [128, NP, PW, PW], BF16, isOutput=False)
        for k in range(3)
    ]
    wk_in = nc.declare_dram_parameter("wk", [128, N_SLOT, 128], BF16, isOutput=False)
    y_out = nc.declare_dram_parameter("y", [128, ZS, DHW, DHW], F32, isOutput=True)

    with tile.TileContext(nc) as tc:
        with (
            tc.tile_pool(name="xw", bufs=1) as xw_pool,
            tc.tile_pool(name="ps", bufs=4, space="PSUM") as ps_pool,
            tc.tile_pool(name="ob", bufs=3) as ob_pool,
        ):

            def body(_iv=None):
                W = xw_pool.tile([128, N_SLOT, 128], BF16, name="W")
                nc.sync.dma_start(out=W[:], in_=wk_in[:])
                # per set: two overlapping 8-plane chunks
                SA = [xw_pool.tile([128, 8, PW, PW], BF16, name=f"SA{k}") for k in range(3)]
                SB = [xw_pool.tile([128, 8, PW, PW], BF16, name=f"SB{k}") for k in range(3)]
                for k in range(3):
                    nc.sync.dma_start(out=SA[k][:], in_=s_in[k][:, 0:8])
                for k in range(3):
                    nc.sync.dma_start(out=SB[k][:], in_=s_in[k][:, 6:14])

                # output tiles: ("plane", chunk, zoff, z) N=504 (21x24, 2D AP)
                #           or ("rem", chunk, zoff, None) N=432 (6x3x24, 3D)
                tiles = (
                    [("plane", SA, 0, z) for z in range(6)]
                    + [("rem", SA, 0, None)]
                    + [("plane", SB, 6, z) for z in range(6, 12)]
                    + [("rem", SB, 6, None)]
                )

                def rhs_ap(X, zoff, kind, z, dz, dy, dx):
                    if kind == "plane":
                        return X[:, z - zoff + dz, dy : dy + 21, dx : dx + 24]
                    return X[:, dz : dz + 6, 21 + dy : 24 + dy, dx : dx + 24]

                def evac(kind, zoff, z, ps, n):
                    ob = ob_pool.tile([128, 512], F32, name="ob", tag="ob")
                    nc.scalar.copy(ob[:, :n], ps[:, :n])
                    if kind == "plane":
                        nc.sync.dma_start(out=y_out[:, z, 0:21, :], in_=ob[:, :n])
                    else:
                        # one DMA per z-plane: keeps each transfer one
                        # contiguous run per partition (descriptor-lean)
                        for j in range(6):
                            nc.sync.dma_start(
                                out=y_out[:, zoff + j, 21:24, :],
                                in_=ob[:, j * 72 : (j + 1) * 72],
                            )

                # process tiles two at a time, alternating the PSUM bank
                # between consecutive matmuls: PSUM accumulation is a
                # read-modify-write that only sustains 1 column / 2 cycles
                # per bank, so a single-bank matmul chain runs at half the
                # PE streaming rate; ping-ponging two banks hides it.
                for t in range(0, len(tiles), 2):
                    pair = tiles[t : t + 2]
                    ns = [504 if kind == "plane" else 432 for kind, _, _, _ in pair]
                    pss = [
                        ps_pool.tile([128, 512], F32, name="ps", tag=f"ps{j}")
                        for j in range(len(pair))
                    ]
                    for s, (si, (dz, dy, dx), _lo, _hi) in enumerate(PAIRS):
                        for j, (kind, S, zoff, z) in enumerate(pair):
                            nc.tensor.matmul(
                                pss[j][:, : ns[j]],
                                lhsT=W[:, s, :],
                                rhs=rhs_ap(S[si], zoff, kind, z, dz, dy, dx),
                                start=(s == 0),
                                stop=(s == N_SLOT - 1),
                                skip_group_check=True,
                            )
                    for j, (kind, S, zoff, z) in enumerate(pair):
                        evac(kind, zoff, z, pss[j], ns[j])

            if loop_n is not None:
                with tc.For_i(0, loop_n, 1) as _i:
                    body(_i)
            else:
                body()

    nc.finalize()
    return nc


def _make_in_maps(x, weight):
    import ml_dtypes

    BF16 = ml_dtypes.bfloat16
    w = np.asarray(weight, np.float32).reshape(COUT, CIN, K, K, K)
    wk = np.zeros((128, N_SLOT, 128), BF16)
    for s, (_si, _off, lo, hi) in enumerate(PAIRS):
        wk[0:64, s, :] = w[:, :, lo[0], lo[1], lo[2]].T.astype(BF16)
        if hi is not None:
            wk[64:128, s, :] = w[:, :, hi[0], hi[1], hi[2]].T.astype(BF16)

    # upper-half shifts per buffer set
    SHIFTS = [(0, 0, 1), (0, 1, 0), (1, 0, 0)]

    in_maps = []
    for c in range(N_CORES):
        b, zh = divmod(c, 2)
        z0 = zh * ZS
        # 27^3 pad so +1 shifts stay in range (extra zero plane at 26)
        xpad = np.zeros((CIN, 27, 27, 27), BF16)
        xpad[:, 1:25, 1:25, 1:25] = x[b].astype(BF16)
        lo_win = xpad[:, z0 : z0 + NP, 0:PW, 0:PW]  # (64, 14, 26, 26)
        im = {"wk": wk}
        for k, (sz, sy, sx) in enumerate(SHIFTS):
            S = np.empty((128, NP, PW, PW), BF16)
            S[0:64] = lo_win
            S[64:128] = xpad[
                :, z0 + sz : z0 + sz + NP, sy : sy + PW, sx : sx + PW
            ]
            im[f"s{k}"] = S
        in_maps.append(im)
    return in_maps


def _gather(results):
    out = np.empty((4, COUT, DHW, DHW, DHW), np.float32)
    for c in range(N_CORES):
        b, zh = divmod(c, 2)
        out[b, :, zh * ZS : (zh + 1) * ZS] = results[c]["y"]
    return out


def kernel(x, weight):
    from concourse.bass_utils import run_bass_kernel_spmd

    x = np.asarray(x, np.float32)
    in_maps = _make_in_maps(x, weight)
    nc = _build_program()
    res = run_bass_kernel_spmd(nc, in_maps, list(range(N_CORES)))
    return _gather(res.results)
